# revision 1
# baseline (speedup 1.0000x reference)
"""Trainium2 Bass kernel for block-local (sparse) attention.

Problem: B=4, T=4096, C=1024, H=16, hd=64, BLOCK_SIZE=256.
  qkv = x @ Wqkv + bqkv ; block-diagonal attention per (batch, head, block)
  out = attn_out @ Wout + bout

Strategy (8 NeuronCores, data parallel over the 64 token blocks):
  - Core i handles 8 consecutive 256-token blocks (2048 tokens), processed as
    4 chunks of 512 tokens so the projection matmuls run at N=512 (f32r
    matmuls only hide their internal weight load at moving-dim >= ~512).
  - Everything on-chip is transposed (feature-on-partition): the host feeds
    x^T and takes y^T back, so no on-device transposes exist at all.
  - All matmuls run in float32r (full PE rate, ~1.5e-4 rel err).
  - Scores are computed as scoresT[j,i] (key-index on partitions); exp is
    taken without max subtraction (scores ~N(0, 0.17), safe); the softmax
    denominator is folded into the o-matmul as a trailing ones-column of the
    v operand (row 64 of the o psum = denominator), then: DVE cross-quadrant
    copy down -> reciprocal -> gpsimd partition-broadcast -> one DVE multiply
    (odd heads write cross-quadrant into lanes 64:127 of the K-tile).
  - Weight layouts are pre-packed on the host so every DMA is wide and
    contiguous; q-scale (hd^-0.5) folds into Wq; the v-bias folds into the
    output bias (softmax rows sum to 1). wqk streams per chunk (SBUF budget);
    wv/wout stay resident.
"""
import numpy as np

import concourse.bass as bass
import concourse.mybir as mybir
import concourse.tile as tile
from concourse import bacc

P = 128
B, T, C = 4, 4096, 1024
H = 16
HD = 64
BS = 256                    # attention block size
NB_TOTAL = (B * T) // BS    # 64 blocks total
N_CORES = 8
NB = NB_TOTAL // N_CORES    # 8 blocks per core
TOK = NB * BS               # 2048 tokens per core
KT = C // P                 # 8 contraction tiles
NPAIR = H // 2              # 8 head pairs
TCH = 512                   # projection chunk (2 blocks)
NCH = TOK // TCH            # 4 chunks per core

f32 = mybir.dt.float32
f32r = mybir.dt.float32r
bf16 = mybir.dt.bfloat16
ATT_DT = f32r   # attention operand dtype: f32r (accurate) or bf16 (fast)


def _build(reps: int = 1, variant: str = 'full'):
    nc = bacc.Bacc(None)

    # x^T pre-tiled: [128, KT, NCH, TCH]
    xT = nc.dram_tensor("xT", [P, KT * NCH * TCH], f32r, kind="ExternalInput")
    # wqk packed m-major for streaming: free = (m*KT + k)*128 + j
    wqk = nc.dram_tensor("wqk", [P, 16 * KT * P], f32r, kind="ExternalInput")
    # wv packed: free = k*1024 + (64h + d)
    wv = nc.dram_tensor("wv", [P, KT * C], f32r, kind="ExternalInput")
    # wout packed: free = (k*8 + t)*128 + e
    wout = nc.dram_tensor("wout", [P, KT * 8 * P], f32r, kind="ExternalInput")
    bqk = nc.dram_tensor("bqk", [P, 16], f32, kind="ExternalInput")
    bout = nc.dram_tensor("bout", [P, 8], f32, kind="ExternalInput")
    # y^T: free = (t_etile*NCH + c)*TCH + i
    yT = nc.dram_tensor("yT", [P, 8 * NCH * TCH], f32, kind="ExternalOutput")

    with tile.TileContext(nc) as tc:
        with (
            tc.tile_pool(name="wpool", bufs=1) as wpool,
            tc.tile_pool(name="wqkpool", bufs=5) as wqkpool,
            tc.tile_pool(name="xpool", bufs=2) as xpool,
            tc.tile_pool(name="qkpool", bufs=16) as qkpool,
            tc.tile_pool(name="vpool", bufs=5) as vpool,
            tc.tile_pool(name="epool", bufs=5) as epool,
            tc.tile_pool(name="rpool", bufs=4) as rpool,
            tc.tile_pool(name="opool", bufs=8) as opool,
            tc.tile_pool(name="ypool", bufs=2) as ypool,
            tc.tile_pool(name="pbig", bufs=2, space="PSUM") as pbig,
            tc.tile_pool(name="psc", bufs=4, space="PSUM") as psc,
            tc.tile_pool(name="ppo", bufs=2, space="PSUM") as ppo,
        ):
            xT_r = xT[:].rearrange("p (k c n) -> p k c n", k=KT, c=NCH)
            wqk_r = wqk[:].rearrange("p (m rest) -> p m rest", m=16)

            # --- prologue: chunk-0 x first, then small consts, then weights ---
            if reps == 1:
                xt0 = xpool.tile([P, KT * TCH], f32r, tag="x")
                nc.sync.dma_start(out=xt0[:].rearrange("p (k n) -> p k n", k=KT),
                                  in_=xT_r[:, :, 0, :])
            bqk_t = wpool.tile([P, 16], f32)
            nc.sync.dma_start(out=bqk_t[:], in_=bqk[:])
            bout_t = wpool.tile([P, 8], f32)
            nc.sync.dma_start(out=bout_t[:], in_=bout[:])
            ones_f = wpool.tile([P, 16], f32)
            nc.vector.memset(ones_f[:], 1.0)
            ones16 = wpool.tile([P, 16], ATT_DT)
            nc.vector.tensor_copy(ones16[:], ones_f[:])

            # chunk-0 wqk prefetch BEFORE the big resident weight DMAs
            # (only for reps==1; cross-loop tile reuse deadlocks under For_i)
            wqk0 = []
            if reps == 1:
                for m in range(16):
                    wm = wqkpool.tile([P, KT * P], f32r, tag="wqk", name=f"wqk0_{m}")
                    nc.sync.dma_start(out=wm[:], in_=wqk_r[:, m, :])
                    wqk0.append(wm)

            wv_t = wpool.tile([P, KT * C], f32r)
            for k in range(KT):
                nc.sync.dma_start(out=wv_t[:, k * C:(k + 1) * C],
                                  in_=wv[:, k * C:(k + 1) * C])
            wout_t = wpool.tile([P, KT * 8 * P], f32r)
            for k in range(KT):
                nc.sync.dma_start(out=wout_t[:, k * 8 * P:(k + 1) * 8 * P],
                                  in_=wout[:, k * 8 * P:(k + 1) * 8 * P])

            def chunk_body(c):
                # 0. x^T chunk [128, KT*512]
                if c == 0 and reps == 1:
                    xt = xt0
                else:
                    xt = xpool.tile([P, KT * TCH], f32r, tag="x")
                    nc.sync.dma_start(
                        out=xt[:].rearrange("p (k n) -> p k n", k=KT),
                        in_=xT_r[:, :, c, :])
                # 1. qk projection: 16 m-tiles, N=512; wqk streamed per m-tile
                qk = []
                for m in range(16):
                    if c == 0 and reps == 1:
                        wm = wqk0[m]
                    else:
                        wm = wqkpool.tile([P, KT * P], f32r, tag="wqk")
                        nc.sync.dma_start(out=wm[:], in_=wqk_r[:, m, :])
                    pt = pbig.tile([P, TCH], f32, tag="big")
                    for k in range(KT):
                        nc.tensor.matmul(
                            pt[:], wm[:, k * P:(k + 1) * P],
                            xt[:, k * TCH:(k + 1) * TCH],
                            start=(k == 0), stop=(k == KT - 1))
                    st = qkpool.tile([P, TCH], ATT_DT, tag="qk")
                    nc.scalar.activation(st[:], pt[:],
                                         mybir.ActivationFunctionType.Identity,
                                         bias=bqk_t[:, m:m + 1])
                    qk.append(st)
                # 2. v projection into v65 tiles [128, 16*65] (ones col per head)
                vt = []
                for ts in range(4):
                    v_sb = vpool.tile([P, 16 * 65], ATT_DT, tag="v")
                    for dch in range(2):
                        pt = pbig.tile([P, 512], f32, tag="big")
                        for k in range(KT):
                            nc.tensor.matmul(
                                pt[:],
                                xt[:, k * TCH + ts * P: k * TCH + (ts + 1) * P],
                                wv_t[:, k * C + dch * 512: k * C + (dch + 1) * 512],
                                start=(k == 0), stop=(k == KT - 1))
                        nc.vector.tensor_copy(
                            v_sb[:, dch * 8 * 65:(dch + 1) * 8 * 65]
                            .rearrange("p (h cc) -> p h cc", h=8)[:, :, 0:HD],
                            pt[:].rearrange("p (h cc) -> p h cc", h=8))
                    nc.vector.tensor_copy(
                        v_sb[:].rearrange("p (h cc) -> p h cc", h=16)[:, :, HD:65],
                        ones16[:].unsqueeze(2))
                    vt.append(v_sb)
                # 3. attention: 2 blocks x 8 pairs
                on_tiles = [opool.tile([P, TCH], f32r, tag="on", name=f"on_{c}_{kk}")
                            for kk in range(8)]
                if variant == 'noattn':
                    for kk in range(8):
                        nc.vector.tensor_copy(on_tiles[kk][:], qk[kk][:])
                for bl in range(2 if variant != 'noattn' else 0):
                    co = bl * BS    # chunk-local column offset of this block
                    for p_ in range(NPAIR):
                        qt, kt_ = qk[p_], qk[8 + p_]
                        ex = [None, None]
                        pss = [psc.tile([P, 2 * BS], f32, tag="sc", name=f"sc{hh}")
                               for hh in range(2)]
                        for jt in range(2):
                            for hh in range(2):
                                lo, hi = hh * HD, (hh + 1) * HD
                                nc.tensor.matmul(
                                    pss[hh][:, jt * BS:(jt + 1) * BS],
                                    kt_[lo:hi, co + jt * P: co + (jt + 1) * P],
                                    qt[lo:hi, co:co + BS], start=True, stop=True)
                        for hh in range(2):
                            e = epool.tile([P, 2 * BS], ATT_DT, tag="e")
                            nc.scalar.activation(
                                e[:], pss[hh][:], mybir.ActivationFunctionType.Exp)
                            ex[hh] = e
                        for hh in range(2):
                            h = 2 * p_ + hh
                            po = ppo.tile([65, BS], f32, tag="po")
                            for jt in range(2):
                                nc.tensor.matmul(
                                    po[:],
                                    vt[2 * bl + jt][:, h * 65:(h + 1) * 65],
                                    ex[hh][:, jt * BS:(jt + 1) * BS],
                                    start=(jt == 0), stop=(jt == 1))
                            if variant == 'nonorm':
                                nc.vector.tensor_copy(
                                    on_tiles[p_][hh * HD:(hh + 1) * HD, co:co + BS],
                                    po[0:HD, :])
                            else:
                                rcp = rpool.tile([1, BS], f32, tag="rcp")
                                nc.vector.reciprocal(rcp[:], po[64:65, :])
                                rcr = rpool.tile([P, BS], f32, tag="rcr")
                                nc.gpsimd.partition_broadcast(rcr[:], rcp[:])
                                nc.vector.tensor_mul(
                                    on_tiles[p_][hh * HD:(hh + 1) * HD, co:co + BS],
                                    po[0:HD, :], rcr[0:HD, :])
                # 4. out projection, N=512
                for t in range(8):
                    pt = pbig.tile([P, TCH], f32, tag="big")
                    for kk in range(KT):
                        nc.tensor.matmul(
                            pt[:], wout_t[:, (kk * 8 + t) * P:(kk * 8 + t + 1) * P],
                            on_tiles[kk][:], start=(kk == 0), stop=(kk == KT - 1))
                    yt = ypool.tile([P, TCH], f32, tag="y")
                    nc.scalar.activation(yt[:], pt[:],
                                         mybir.ActivationFunctionType.Identity,
                                         bias=bout_t[:, t:t + 1])
                    nc.sync.dma_start(
                        out=yT[:, (t * NCH + c) * TCH:(t * NCH + c + 1) * TCH],
                        in_=yt[:])

            def all_chunks():
                for c in range(NCH):
                    chunk_body(c)

            if reps == 1:
                all_chunks()
            else:
                with tc.For_i(0, reps, 1):
                    all_chunks()
    nc.finalize()
    return nc


def prep_inputs(x, Wqkv, bqkv, Wout, bout):
    """Host-side shard + repack. Returns list of 8 per-core input dicts."""
    x = np.asarray(x, dtype=np.float32)
    Wqkv = np.asarray(Wqkv, dtype=np.float32)
    bqkv = np.asarray(bqkv, dtype=np.float32)
    Wout = np.asarray(Wout, dtype=np.float32)
    bout = np.asarray(bout, dtype=np.float32)

    scale = 1.0 / np.sqrt(HD)
    W3 = Wqkv.reshape(C, H, 3 * HD)
    b3 = bqkv.reshape(H, 3 * HD)
    Wq = W3[:, :, 0:HD] * scale          # [C, H, 64]
    Wk = W3[:, :, HD:2 * HD]
    Wv = W3[:, :, 2 * HD:3 * HD]
    bq = b3[:, 0:HD] * scale
    bk = b3[:, HD:2 * HD]
    bv = b3[:, 2 * HD:3 * HD]

    # m-tiles: m<8 -> [Wq_{2m} | Wq_{2m+1}], m>=8 -> k-pairs
    mt = np.empty((C, 16, P), dtype=np.float32)
    for m in range(8):
        mt[:, m, 0:HD] = Wq[:, 2 * m]
        mt[:, m, HD:P] = Wq[:, 2 * m + 1]
        mt[:, 8 + m, 0:HD] = Wk[:, 2 * m]
        mt[:, 8 + m, HD:P] = Wk[:, 2 * m + 1]
    # -> [128, m, k, 128] m-major flat
    wqk_h = np.ascontiguousarray(
        mt.reshape(KT, P, 16, P).transpose(1, 2, 0, 3).reshape(P, 16 * KT * P))

    wv_full = Wv.reshape(C, H * HD)
    wv_h = np.ascontiguousarray(
        wv_full.reshape(KT, P, C).transpose(1, 0, 2).reshape(P, KT * C))

    wout_h = np.ascontiguousarray(
        Wout.reshape(KT, P, 8, P).transpose(1, 0, 2, 3).reshape(P, KT * 8 * P))

    bqk_h = np.empty((P, 16), dtype=np.float32)
    for m in range(8):
        bqk_h[0:HD, m] = bq[2 * m]
        bqk_h[HD:P, m] = bq[2 * m + 1]
        bqk_h[0:HD, 8 + m] = bk[2 * m]
        bqk_h[HD:P, 8 + m] = bk[2 * m + 1]

    boutp = bout + bv.reshape(H * HD) @ Wout
    bout_h = np.ascontiguousarray(boutp.reshape(8, P).T)

    xb = x.reshape(NB_TOTAL, BS, C)
    in_maps = []
    for core in range(N_CORES):
        blocks = xb[core * NB:(core + 1) * NB]
        xTc = blocks.reshape(TOK, C).T                  # [C, 2048]
        xTt = (xTc.reshape(KT, P, NCH, TCH)
               .transpose(1, 0, 2, 3).reshape(P, KT * NCH * TCH))
        in_maps.append({
            "xT": np.ascontiguousarray(xTt),
            "wqk": wqk_h, "wv": wv_h, "wout": wout_h,
            "bqk": bqk_h, "bout": bout_h,
        })
    return in_maps


def assemble_output(results):
    """results: list of 8 dicts with 'yT' [128, 8*NCH*TCH] -> full y [B, T, C]."""
    y = np.empty((N_CORES, TOK, C), dtype=np.float32)
    for core, r in enumerate(results):
        yT = r["yT"].reshape(P, 8, NCH, TCH)   # [p, etile, c, i]
        yc = yT.transpose(2, 3, 1, 0).reshape(TOK, C)
        y[core] = yc
    return y.reshape(B, T, C)


_CACHED = {}


def kernel(x, Wqkv, bqkv, Wout, bout):
    from concourse.bass_utils import run_bass_kernel_spmd
    if "nc" not in _CACHED:
        _CACHED["nc"] = _build(reps=1)
    in_maps = prep_inputs(x, Wqkv, bqkv, Wout, bout)
    res = run_bass_kernel_spmd(_CACHED["nc"], in_maps, list(range(N_CORES)))
    return assemble_output(res.results)



# revision 14
# speedup vs baseline: 2.6196x; 2.6196x over previous
"""Trainium2 Bass kernel for block-local (sparse) attention.

Problem: B=4, T=4096, C=1024, H=16, hd=64, BLOCK_SIZE=256.
  qkv = x @ Wqkv + bqkv ; block-diagonal attention per (batch, head, block)
  out = attn_out @ Wout + bout

Strategy (8 NeuronCores, data parallel over the 64 token blocks):
  - Core i handles 8 consecutive 256-token blocks (2048 tokens), processed as
    4 chunks of 512 tokens (projection matmuls at N=512).
  - Everything on-chip is transposed (feature-on-partition): the host feeds
    x^T and takes y^T back, so no on-device transposes exist at all.
  - All operands are bf16 (same 1 cycle/row PE rate as f32r, half the DMA/
    SBUF/elementwise cost); PSUM accumulation stays f32.
  - All weights are SBUF-resident (loaded once in the prologue).
  - Softmax without max-subtraction (scores ~N(0,0.17), safe). Denominators
    for all 16 heads of a chunk are accumulated into one PSUM tile [16,512]
    via ones-vector matmuls on PE, reciprocal'd in one batched DVE op, then
    broadcast to [128,512] per head-pair with a constant selection-matrix
    matmul on PE. Normalization folds into the PSUM->SBUF evacuation of the
    attention output as one [128,256] tensor_mul per (block,pair), split
    between DVE and Pool. (No gpsimd partition_broadcast anywhere.)
  - Chunk phases are software-pipelined: proj(c+1) is emitted between
    attnA(c) (scores/exp/den) and attnB(c) (o-matmul/normalize) so PE never
    stalls on the reciprocal.
"""
import numpy as np

import concourse.bass as bass
import concourse.mybir as mybir
import concourse.tile as tile
from concourse import bacc

P = 128
B, T, C = 4, 4096, 1024
H = 16
HD = 64
BS = 256                    # attention block size
NB_TOTAL = (B * T) // BS    # 64 blocks total
N_CORES = 8
NB = NB_TOTAL // N_CORES    # 8 blocks per core
TOK = NB * BS               # 2048 tokens per core
KT = C // P                 # 8 contraction tiles
NPAIR = H // 2              # 8 head pairs
TCH = 512                   # projection chunk (2 blocks)
NCH = TOK // TCH            # 4 chunks per core

f32 = mybir.dt.float32
f32r = mybir.dt.float32r
bf16 = mybir.dt.bfloat16


def _build(reps: int = 1, variant: str = 'full'):
    nc = bacc.Bacc(None)

    # x^T pre-tiled: [128, KT, NCH, TCH] bf16
    xT = nc.dram_tensor("xT", [P, KT * NCH * TCH], bf16, kind="ExternalInput")
    # wqk packed m-major: free = (m*KT + k)*128 + j
    wqk = nc.dram_tensor("wqk", [P, 16 * KT * P], bf16, kind="ExternalInput")
    # wv packed: free = k*1024 + (64h + d)
    wv = nc.dram_tensor("wv", [P, KT * C], bf16, kind="ExternalInput")
    # wout packed: free = (k*8 + t)*128 + e
    wout = nc.dram_tensor("wout", [P, KT * 8 * P], bf16, kind="ExternalInput")
    bqk = nc.dram_tensor("bqk", [P, 16], f32, kind="ExternalInput")
    bout = nc.dram_tensor("bout", [P, 8], f32, kind="ExternalInput")
    # constant selection matrices (see prep_inputs)
    sel = nc.dram_tensor("sel", [16, NPAIR * P], f32r, kind="ExternalInput")
    onehd = nc.dram_tensor("onehd", [P, 16 * 16], bf16, kind="ExternalInput")
    # y^T: free = (t_etile*NCH + c)*TCH + i
    yT = nc.dram_tensor("yT", [P, 8 * NCH * TCH], f32, kind="ExternalOutput")

    with tile.TileContext(nc) as tc:
        with (
            tc.tile_pool(name="wpool", bufs=1) as wpool,
            tc.tile_pool(name="xpool", bufs=2) as xpool,
            tc.tile_pool(name="qkpool", bufs=24) as qkpool,
            tc.tile_pool(name="vpool", bufs=9) as vpool,
            tc.tile_pool(name="epool", bufs=35) as epool,
            tc.tile_pool(name="rpool", bufs=2) as rpool,
            tc.tile_pool(name="opool", bufs=10) as opool,
            tc.tile_pool(name="ypool", bufs=2) as ypool,
            tc.tile_pool(name="pbig", bufs=2, space="PSUM") as pbig,
            tc.tile_pool(name="pmix", bufs=3, space="PSUM") as pmix,
            tc.tile_pool(name="pden", bufs=1, space="PSUM") as pden,
            tc.tile_pool(name="ppo", bufs=2, space="PSUM") as ppo,
        ):
            xT_r = xT[:].rearrange("p (k c n) -> p k c n", k=KT, c=NCH)

            # --- prologue: chunk-0 x first, then consts, then weights ---
            if reps == 1:
                xt0 = xpool.tile([P, KT * TCH], bf16, tag="x")
                nc.sync.dma_start(out=xt0[:].rearrange("p (k n) -> p k n", k=KT),
                                  in_=xT_r[:, :, 0, :])
            bqk_t = wpool.tile([P, 16], f32)
            nc.sync.dma_start(out=bqk_t[:], in_=bqk[:])
            bout_t = wpool.tile([P, 8], f32)
            nc.sync.dma_start(out=bout_t[:], in_=bout[:])
            # one-hot column tiles: oneh[:, h*16+h] = 1 — stationary for the
            # denominator-accumulation matmuls (adds e-column-sums into den
            # row h, zero elsewhere; base-partition rules forbid writing at
            # partition h directly)
            oneh = wpool.tile([P, 16 * 16], bf16)
            nc.sync.dma_start(out=oneh[:], in_=onehd[:])
            # selection matrices: S8[:, p*128:(p+1)*128] maps den rows
            # (2p, 2p+1) onto partitions 0:64 / 64:128
            S8 = wpool.tile([16, NPAIR * P], f32r)
            nc.sync.dma_start(out=S8[:], in_=sel[:])

            wqk_t = wpool.tile([P, 16 * KT * P], bf16)
            for m in range(16):
                nc.sync.dma_start(out=wqk_t[:, m * KT * P:(m + 1) * KT * P],
                                  in_=wqk[:, m * KT * P:(m + 1) * KT * P])
            wv_t = wpool.tile([P, KT * C], bf16)
            for k in range(KT):
                nc.sync.dma_start(out=wv_t[:, k * C:(k + 1) * C],
                                  in_=wv[:, k * C:(k + 1) * C])
            wout_t = wpool.tile([P, KT * 8 * P], bf16)
            for k in range(KT):
                nc.sync.dma_start(out=wout_t[:, k * 8 * P:(k + 1) * 8 * P],
                                  in_=wout[:, k * 8 * P:(k + 1) * 8 * P])

            def proj(c):
                """qk + v projections for chunk c -> (qk tiles, v tiles)."""
                if c == 0 and reps == 1:
                    xt = xt0
                else:
                    xt = xpool.tile([P, KT * TCH], bf16, tag="x")
                    nc.sync.dma_start(
                        out=xt[:].rearrange("p (k n) -> p k n", k=KT),
                        in_=xT_r[:, :, c, :])
                qk = []
                for m in range(16):
                    pt = pbig.tile([P, TCH], f32, tag="big")
                    for k in range(KT):
                        nc.tensor.matmul(
                            pt[:],
                            wqk_t[:, (m * KT + k) * P:(m * KT + k + 1) * P],
                            xt[:, k * TCH:(k + 1) * TCH],
                            start=(k == 0), stop=(k == KT - 1))
                    st = qkpool.tile([P, TCH], bf16, tag="qk")
                    nc.scalar.activation(st[:], pt[:],
                                         mybir.ActivationFunctionType.Identity,
                                         bias=bqk_t[:, m:m + 1])
                    qk.append(st)
                vt = []
                for ts in range(4):
                    v_sb = vpool.tile([P, C], bf16, tag="v")
                    for dch in range(2):
                        pt = pbig.tile([P, 512], f32, tag="big")
                        for k in range(KT):
                            nc.tensor.matmul(
                                pt[:],
                                xt[:, k * TCH + ts * P: k * TCH + (ts + 1) * P],
                                wv_t[:, k * C + dch * 512: k * C + (dch + 1) * 512],
                                start=(k == 0), stop=(k == KT - 1))
                        nc.vector.tensor_copy(
                            v_sb[:, dch * 512:(dch + 1) * 512], pt[:])
                    vt.append(v_sb)
                return qk, vt

            def attnA(c, qk):
                """scores + exp + denominator accumulation for chunk c."""
                ex = {}
                den_t = pden.tile([16, TCH], f32, tag="den")
                for bl in range(2):
                    co = bl * BS
                    for p_ in range(NPAIR):
                        qt, kt_ = qk[p_], qk[8 + p_]
                        for hh in range(2):
                            lo, hi = hh * HD, (hh + 1) * HD
                            pss = pmix.tile([P, 2 * BS], f32, tag="mix")
                            for jt in range(2):
                                nc.tensor.matmul(
                                    pss[:, jt * BS:(jt + 1) * BS],
                                    kt_[lo:hi, co + jt * P: co + (jt + 1) * P],
                                    qt[lo:hi, co:co + BS],
                                    start=True, stop=True)
                            e = epool.tile([P, 2 * BS], bf16, tag="e")
                            nc.scalar.activation(
                                e[:], pss[:], mybir.ActivationFunctionType.Exp)
                            ex[(bl, p_, hh)] = e
                            h = 2 * p_ + hh
                            for jt in range(2):
                                nc.tensor.matmul(
                                    den_t[:, co:co + BS],
                                    oneh[:, h * 16:(h + 1) * 16],
                                    e[:, jt * BS:(jt + 1) * BS],
                                    start=(p_ == 0 and hh == 0 and jt == 0),
                                    stop=(p_ == NPAIR - 1 and hh == 1
                                          and jt == 1))
                return ex, den_t

            def attnB(c, vt, ex, den_t):
                """reciprocal, broadcast, o-matmuls, normalized evac."""
                on_tiles = [opool.tile([P, TCH], bf16, tag="on",
                                       name=f"on_{c}_{kk}")
                            for kk in range(8)]
                rden = rpool.tile([16, TCH], f32r, tag="rden")
                # f32r is bit-identical to f32; only the matmul replay mode
                # differs, so this is not actually a precision loss
                with nc.allow_low_precision(reason="f32r == f32 bits"):
                    nc.vector.reciprocal(rden[:], den_t[:])
                for p_ in range(NPAIR):
                    rt = pmix.tile([P, TCH], f32, tag="mix")
                    nc.tensor.matmul(rt[:], S8[:, p_ * P:(p_ + 1) * P],
                                     rden[:], start=True, stop=True)
                    # DVE can read only one PSUM operand per op (and Pool
                    # none), so stage the broadcast reciprocals through SBUF
                    rt_sb = rpool.tile([P, TCH], f32, tag="rtsb")
                    nc.scalar.copy(rt_sb[:], rt[:])
                    for bl in range(2):
                        co = bl * BS
                        po2 = ppo.tile([P, BS], f32, tag="po")
                        for hh in range(2):
                            h = 2 * p_ + hh
                            for jt in range(2):
                                nc.tensor.matmul(
                                    po2[hh * HD:(hh + 1) * HD, :],
                                    vt[2 * bl + jt][:, h * HD:(h + 1) * HD],
                                    ex[(bl, p_, hh)][:, jt * BS:(jt + 1) * BS],
                                    start=(jt == 0), stop=(jt == 1))
                        nc.vector.tensor_mul(on_tiles[p_][:, co:co + BS],
                                             po2[:], rt_sb[:, co:co + BS])
                return on_tiles

            def attn_skip(c, qk):
                on_tiles = [opool.tile([P, TCH], bf16, tag="on",
                                       name=f"on_{c}_{kk}")
                            for kk in range(8)]
                for kk in range(8):
                    nc.vector.tensor_copy(on_tiles[kk][:], qk[kk][:])
                return on_tiles

            def attnB_nonorm(c, vt, ex, den_t):
                on_tiles = [opool.tile([P, TCH], bf16, tag="on",
                                       name=f"on_{c}_{kk}")
                            for kk in range(8)]
                for p_ in range(NPAIR):
                    for bl in range(2):
                        co = bl * BS
                        po2 = ppo.tile([P, BS], f32, tag="po")
                        for hh in range(2):
                            h = 2 * p_ + hh
                            for jt in range(2):
                                nc.tensor.matmul(
                                    po2[hh * HD:(hh + 1) * HD, :],
                                    vt[2 * bl + jt][:, h * HD:(h + 1) * HD],
                                    ex[(bl, p_, hh)][:, jt * BS:(jt + 1) * BS],
                                    start=(jt == 0), stop=(jt == 1))
                        nc.vector.tensor_copy(on_tiles[p_][:, co:co + BS],
                                              po2[:])
                return on_tiles

            def outproj(c, on_tiles):
                for t in range(8):
                    pt = pbig.tile([P, TCH], f32, tag="big")
                    for kk in range(KT):
                        nc.tensor.matmul(
                            pt[:],
                            wout_t[:, (kk * 8 + t) * P:(kk * 8 + t + 1) * P],
                            on_tiles[kk][:], start=(kk == 0), stop=(kk == KT - 1))
                    yt = ypool.tile([P, TCH], f32, tag="y")
                    nc.scalar.activation(yt[:], pt[:],
                                         mybir.ActivationFunctionType.Identity,
                                         bias=bout_t[:, t:t + 1])
                    nc.sync.dma_start(
                        out=yT[:, (t * NCH + c) * TCH:(t * NCH + c + 1) * TCH],
                        in_=yt[:])

            def all_chunks():
                if variant == 'noattn':
                    for c in range(NCH):
                        qk, vt = proj(c)
                        outproj(c, attn_skip(c, qk))
                    return
                # software pipeline: proj(c+1) sits between A(c) and B(c)
                qk, vt = proj(0)
                ex, den_t = attnA(0, qk)
                state = (vt, ex, den_t)
                for c in range(NCH):
                    nxt = proj(c + 1) if c + 1 < NCH else None
                    vt, ex, den_t = state
                    if variant == 'nonorm':
                        on_tiles = attnB_nonorm(c, vt, ex, den_t)
                    else:
                        on_tiles = attnB(c, vt, ex, den_t)
                    outproj(c, on_tiles)
                    if nxt is not None:
                        qk2, vt2 = nxt
                        ex2, den2 = attnA(c + 1, qk2)
                        state = (vt2, ex2, den2)

            if reps == 1:
                all_chunks()
            else:
                with tc.For_i(0, reps, 1):
                    all_chunks()
    nc.finalize()
    return nc


def prep_inputs(x, Wqkv, bqkv, Wout, bout):
    """Host-side shard + repack. Returns list of 8 per-core input dicts."""
    np_bf16 = mybir.dt.np(bf16)
    x = np.asarray(x, dtype=np.float32)
    Wqkv = np.asarray(Wqkv, dtype=np.float32)
    bqkv = np.asarray(bqkv, dtype=np.float32)
    Wout = np.asarray(Wout, dtype=np.float32)
    bout = np.asarray(bout, dtype=np.float32)

    scale = 1.0 / np.sqrt(HD)
    W3 = Wqkv.reshape(C, H, 3 * HD)
    b3 = bqkv.reshape(H, 3 * HD)
    Wq = W3[:, :, 0:HD] * scale          # [C, H, 64]
    Wk = W3[:, :, HD:2 * HD]
    Wv = W3[:, :, 2 * HD:3 * HD]
    bq = b3[:, 0:HD] * scale
    bk = b3[:, HD:2 * HD]
    bv = b3[:, 2 * HD:3 * HD]

    # m-tiles: m<8 -> [Wq_{2m} | Wq_{2m+1}], m>=8 -> k-pairs
    mt = np.empty((C, 16, P), dtype=np.float32)
    for m in range(8):
        mt[:, m, 0:HD] = Wq[:, 2 * m]
        mt[:, m, HD:P] = Wq[:, 2 * m + 1]
        mt[:, 8 + m, 0:HD] = Wk[:, 2 * m]
        mt[:, 8 + m, HD:P] = Wk[:, 2 * m + 1]
    # -> [128, m, k, 128] m-major flat
    wqk_h = np.ascontiguousarray(
        mt.reshape(KT, P, 16, P).transpose(1, 2, 0, 3)
        .reshape(P, 16 * KT * P)).astype(np_bf16)

    wv_full = Wv.reshape(C, H * HD)
    wv_h = np.ascontiguousarray(
        wv_full.reshape(KT, P, C).transpose(1, 0, 2)
        .reshape(P, KT * C)).astype(np_bf16)

    wout_h = np.ascontiguousarray(
        Wout.reshape(KT, P, 8, P).transpose(1, 0, 2, 3)
        .reshape(P, KT * 8 * P)).astype(np_bf16)

    bqk_h = np.empty((P, 16), dtype=np.float32)
    for m in range(8):
        bqk_h[0:HD, m] = bq[2 * m]
        bqk_h[HD:P, m] = bq[2 * m + 1]
        bqk_h[0:HD, 8 + m] = bk[2 * m]
        bqk_h[HD:P, 8 + m] = bk[2 * m + 1]

    boutp = bout + bv.reshape(H * HD) @ Wout
    bout_h = np.ascontiguousarray(boutp.reshape(8, P).T)

    sel_h = np.zeros((16, NPAIR * P), dtype=np.float32)
    for p_ in range(NPAIR):
        sel_h[2 * p_, p_ * P:p_ * P + HD] = 1.0
        sel_h[2 * p_ + 1, p_ * P + HD:(p_ + 1) * P] = 1.0
    onehd_h = np.zeros((P, 16 * 16), dtype=np_bf16)
    for h in range(16):
        onehd_h[:, h * 16 + h] = 1.0

    xb = x.reshape(NB_TOTAL, BS, C)
    in_maps = []
    for core in range(N_CORES):
        blocks = xb[core * NB:(core + 1) * NB]
        xTc = blocks.reshape(TOK, C).T                  # [C, 2048]
        xTt = (xTc.reshape(KT, P, NCH, TCH)
               .transpose(1, 0, 2, 3).reshape(P, KT * NCH * TCH))
        in_maps.append({
            "xT": np.ascontiguousarray(xTt).astype(np_bf16),
            "wqk": wqk_h, "wv": wv_h, "wout": wout_h,
            "bqk": bqk_h, "bout": bout_h,
            "sel": sel_h, "onehd": onehd_h,
        })
    return in_maps


def assemble_output(results):
    """results: list of 8 dicts with 'yT' [128, 8*NCH*TCH] -> full y [B, T, C]."""
    y = np.empty((N_CORES, TOK, C), dtype=np.float32)
    for core, r in enumerate(results):
        yT = r["yT"].reshape(P, 8, NCH, TCH)   # [p, etile, c, i]
        yc = yT.transpose(2, 3, 1, 0).reshape(TOK, C)
        y[core] = yc
    return y.reshape(B, T, C)


_CACHED = {}


def kernel(x, Wqkv, bqkv, Wout, bout):
    from concourse.bass_utils import run_bass_kernel_spmd
    if "nc" not in _CACHED:
        _CACHED["nc"] = _build(reps=1)
    in_maps = prep_inputs(x, Wqkv, bqkv, Wout, bout)
    res = run_bass_kernel_spmd(_CACHED["nc"], in_maps, list(range(N_CORES)))
    return assemble_output(res.results)
